# revision 5
# baseline (speedup 1.0000x reference)
"""Linear attention ("Transformers are RNNs") on 8 Trainium2 NeuronCores.

Problem: N=8, L=S=8192, H=8, D=Dv=32, f32.
    phi(x) = elu(x)+1
    A[d,v] = sum_s phi(K)[s,d] V[s,v]        (the /v_length ... *v_length cancels exactly)
    b[d]   = sum_s phi(K)[s,d]
    out[l,v] = (sum_d phi(Q)[l,d] A[d,v]) / (sum_d phi(Q)[l,d] b[d] + EPS)

Sharding: batch element n -> core n (fully independent, no collectives).

Device layout choices:
  - Q is pre-transposed on the host to [H*D, L] so the contraction dim (d)
    lands on SBUF partitions with perfectly contiguous DMA. No on-device
    transposes anywhere.
  - phi(x) = min(exp(x), 1 + relu(x))  (exactly elu(x)+1):
        e = Exp(x)            (ScalarE)
        t = (x max 0) add 1   (VectorE dual-op tensor_scalar)
        phi = min(e, t)       (VectorE tensor_tensor)
  - Phase 1: per 128-row s-tile, for each 4-head group g, one f32r matmul
        lhsT = phi(K)[:, g*128:(g+1)*128]   (s=128 contraction, m=(head,d))
        rhs  = [V(all 256 cols) | ones]     (N=257 -> 1 cycle/row f32r)
    accumulated into PSUM[128, 257] over all 64 s-tiles. Diagonal head
    blocks give A, the ones column gives b. Off-diagonal blocks are junk.
  - Phase 1.5: assemble block-diagonal Baug[128, 4*33] = diag([A_h | b_h]).
  - Phase 2: per 128-row l-tile, per group: one matmul
        out[l, (j, v_aug)] = phiQT_g.T @ Baug_g    (N=132)
    col 33j+32 is the raw denominator. Normalize with reciprocal +
    broadcast multiply, DMA out in the natural [l, h*32+v] layout.
"""

import sys

for _p in ("/opt/trn_rl_repo",):
    if _p not in sys.path:
        sys.path.insert(0, _p)

import numpy as np

from concourse import bacc, bass, mybir, tile
from concourse.bass_utils import run_bass_kernel_spmd

# ---------------------------------------------------------------- constants
N_BATCH = 8
L = 8192
S = 8192
H = 8
D = 32
HD = H * D  # 256
P = 128
EPS = 1e-6

F32 = mybir.dt.float32
F32R = mybir.dt.float32r
BF16 = mybir.dt.bfloat16
AF = mybir.ActivationFunctionType
OP = mybir.AluOpType

MACRO = 4  # 128-row subtiles per macro tile (elementwise ops amortize overhead)
N_SUB = S // P  # 64 subtiles
N_MACRO = N_SUB // MACRO  # 16 macro iterations
QMACRO = 8  # l-subtiles per phase-2 macro
N_QMACRO = L // (P * QMACRO)  # 8

G = 2  # head groups (4 heads each)
VA = HD + 1  # 257: V columns + ones column
BW = 4 * (D + 1)  # 132: block-diag width per group


def _bcast_last(ap, n):
    """Append a stride-0 dim of size n to an AP (free-dim broadcast)."""
    ap = ap.unsqueeze(ap.ndim)
    return ap.broadcast_to(tuple(ap.shape[:-1]) + (n,))


def _phi(nc, pool, x, fd):
    """phi(x) = elu(x)+1 = min(exp(x), 1 + relu(x)); x is [P, fd] f32 SBUF.

    Result is bf16 (matmul operand)."""
    e = pool.tile([P, fd], F32, tag="phi_e")
    t = pool.tile([P, fd], F32, tag="phi_t")
    phi = pool.tile([P, fd], BF16, tag="phi_o")
    nc.scalar.activation(e[:], x[:], AF.Exp)
    nc.vector.tensor_scalar(t[:], x[:], 0.0, 1.0, OP.max, OP.add)
    nc.vector.tensor_tensor(phi[:], e[:], t[:], OP.min)
    return phi


def _build_body(nc, tc, qt, kk, vv, out):
    with (
        tc.tile_pool(name="io", bufs=3) as io,
        tc.tile_pool(name="ew", bufs=2) as ew,
        tc.tile_pool(name="misc", bufs=1) as misc,
        tc.tile_pool(name="small", bufs=4) as small,
        tc.tile_pool(name="outp", bufs=4) as outp,
        tc.tile_pool(name="ps1", bufs=1, space="PSUM") as ps1,
        tc.tile_pool(name="ps2", bufs=4, space="PSUM") as ps2,
    ):
        # ---------------- phase 1: KV/b accumulation over S ----------------
        pacc = [ps1.tile([P, VA], F32, tag=f"pacc{g}", name=f"pacc{g}") for g in range(G)]

        for m in range(N_MACRO):
            k_t = io.tile([P, MACRO * HD], F32, tag="k_t")
            v_t = io.tile([P, MACRO * VA], F32, tag="v_t")

            rows = kk[m * MACRO * P : (m + 1) * MACRO * P, :]
            nc.sync.dma_start(
                k_t[:].rearrange("p (b c) -> p b c", b=MACRO),
                rows.rearrange("(b p) c -> p b c", p=P),
            )
            v_rows = vv[m * MACRO * P : (m + 1) * MACRO * P, :]
            v3 = v_t[:].rearrange("p (b c) -> p b c", b=MACRO)
            nc.sync.dma_start(v3[:, :, 0:HD], v_rows.rearrange("(b p) c -> p b c", p=P))
            nc.gpsimd.memset(v3[:, :, HD : HD + 1], 1.0)
            v_r = io.tile([P, MACRO * VA], BF16, tag="v_r")
            nc.vector.tensor_copy(v_r[:], v_t[:])

            phi = _phi(nc, ew, k_t, MACRO * HD)

            for b in range(MACRO):
                for g in range(G):
                    nc.tensor.matmul(
                        pacc[g][:],
                        phi[:, b * HD + g * P : b * HD + (g + 1) * P],
                        v_r[:, b * VA : (b + 1) * VA],
                        start=(m == 0 and b == 0),
                        stop=(m == N_MACRO - 1 and b == MACRO - 1),
                    )

        # ------------- phase 1.5: block-diagonal [A_h | b_h] weights -------
        baug = []
        for g in range(G):
            bg = misc.tile([P, BW], BF16, tag=f"baug{g}", name=f"baug{g}")
            nc.vector.memset(bg[:], 0.0)
            for j in range(4):
                r0 = 32 * j
                nc.scalar.copy(
                    bg[r0 : r0 + 32, 33 * j : 33 * j + 32],
                    pacc[g][r0 : r0 + 32, g * P + r0 : g * P + r0 + 32],
                )
                nc.scalar.copy(
                    bg[r0 : r0 + 32, 33 * j + 32 : 33 * j + 33],
                    pacc[g][r0 : r0 + 32, HD : HD + 1],
                )
            baug.append(bg)

        # ---------------- phase 2: queries ----------------
        for mq in range(N_QMACRO):
            c0 = mq * QMACRO * P
            phis = []
            for g in range(G):
                qt_t = io.tile([P, QMACRO * P], F32, tag=f"qt{g}", name=f"qt{g}")
                nc.sync.dma_start(qt_t[:], qt[g * P : (g + 1) * P, c0 : c0 + QMACRO * P])
                phis.append(_phi(nc, ew, qt_t, QMACRO * P))

            for i in range(QMACRO):
                o_ps = ps2.tile([P, G * BW], F32, tag="o_ps")
                for g in range(G):
                    nc.tensor.matmul(
                        o_ps[:, g * BW : (g + 1) * BW],
                        phis[g][:, i * P : (i + 1) * P],
                        baug[g][:],
                        start=True,
                        stop=True,
                    )
                o4 = o_ps[:].rearrange("p (g j c) -> p g j c", g=G, c=33)
                den = small.tile([P, G * 4], F32, tag="den")
                rcp = small.tile([P, G * 4], F32, tag="rcp")
                nc.vector.tensor_scalar(
                    den[:].rearrange("p (g j) -> p g j", g=G),
                    o4[:, :, :, 32],
                    EPS,
                    None,
                    OP.add,
                )
                nc.vector.reciprocal(rcp[:], den[:])
                out_t = outp.tile([P, HD], F32, tag="out_t")
                nc.vector.tensor_tensor(
                    out_t[:].rearrange("p (g j c) -> p g j c", g=G, c=32),
                    o4[:, :, :, 0:32],
                    _bcast_last(rcp[:].rearrange("p (g j) -> p g j", g=G), 32),
                    OP.mult,
                )
                r0 = c0 + i * P
                nc.sync.dma_start(out[r0 : r0 + P, :], out_t[:])


_NC_CACHE = None


def build_nc():
    global _NC_CACHE
    if _NC_CACHE is not None:
        return _NC_CACHE
    nc = bacc.Bacc(
        "TRN2",
        target_bir_lowering=False,
        debug=False,
        enable_asserts=False,
        num_devices=N_BATCH,
    )
    qt = nc.dram_tensor("qt", [HD, L], F32, kind="ExternalInput").ap()
    kk = nc.dram_tensor("kk", [S, HD], F32, kind="ExternalInput").ap()
    vv = nc.dram_tensor("vv", [S, HD], F32, kind="ExternalInput").ap()
    out = nc.dram_tensor("out", [L, HD], F32, kind="ExternalOutput").ap()
    with tile.TileContext(nc) as tc:
        _build_body(nc, tc, qt, kk, vv, out)
    nc.compile()
    _NC_CACHE = nc
    return nc


def make_in_maps(queries, keys, values):
    queries = np.asarray(queries, dtype=np.float32)
    keys = np.asarray(keys, dtype=np.float32)
    values = np.asarray(values, dtype=np.float32)
    in_maps = []
    for n in range(N_BATCH):
        qt = np.ascontiguousarray(
            queries[n].transpose(1, 2, 0).reshape(HD, L)
        )  # [h*32+d, l]
        in_maps.append(
            {
                "qt": qt,
                "kk": np.ascontiguousarray(keys[n].reshape(S, HD)),
                "vv": np.ascontiguousarray(values[n].reshape(S, HD)),
            }
        )
    return in_maps


def run(queries, keys, values, trace=False, **kwargs):
    nc = build_nc()
    in_maps = make_in_maps(queries, keys, values)
    res = run_bass_kernel_spmd(
        nc, in_maps, core_ids=list(range(N_BATCH)), trace=trace, **kwargs
    )
    outs = [res.results[n]["out"].reshape(L, H, D) for n in range(N_BATCH)]
    return np.stack(outs, axis=0), res


def kernel(queries, keys, values):
    out, _ = run(queries, keys, values, trace=False)
    return out
